# revision 6
# baseline (speedup 1.0000x reference)
"""Trainium2 Bass kernel for GNN message passing + dense_mincut_pool losses.

Computes, given X [8192,256], A [8192,8192] (symmetric 0/1 with self loops)
and GraphConv/MLP weights:
    S = GraphConv(LayerNorm(X), A-pattern) @ mlp    -> [8192, 32]
    loss_mc, loss_o  (dense_mincut_pool losses on softmax(S))

Distribution: A is sharded column-wise across 8 NeuronCores (core k gets
A[:, k*1024:(k+1)*1024], which by symmetry equals A[rows_k, :]^T). X is
replicated. Each core computes S for its 1024 nodes; softmaxed cluster
assignments are all-gathered for the s^T A s contraction; tiny [C,C]
partials are all-reduced; losses are computed redundantly on every core.
"""

import os
import sys

sys.path.insert(0, "/opt/trn_rl_repo")
os.environ.setdefault("MYCRO_LOCAL_CACHE", "1")

import numpy as np
from contextlib import ExitStack  # noqa: F401

import concourse.bass as bass
import concourse.bacc as bacc
import concourse.mybir as mybir
import concourse.tile as tile
import concourse.bass_isa as bass_isa
from concourse.bass_utils import run_bass_kernel_spmd

N, T, F, C = 8192, 256, 128, 32
NCORES = 8
ML = N // NCORES       # 1024 local nodes per core
KT = N // 128          # 64 contraction tiles
MT = ML // 128         # 8 local node tiles
EPS = 1e-5
FP = mybir.dt.float32

_cache = {}
last_exec_time_ns = None


def _build():
    nc = bacc.Bacc("TRN2", target_bir_lowering=False, debug=False,
                   num_devices=NCORES)

    Ash = nc.declare_dram_parameter("Ash", [N, ML], FP, isOutput=False)
    Xf = nc.declare_dram_parameter("Xf", [N, T], FP, isOutput=False)
    Xloc = nc.declare_dram_parameter("Xloc", [ML, T], FP, isOutput=False)
    WrT = nc.declare_dram_parameter("WrT", [T, F], FP, isOutput=False)
    WqT = nc.declare_dram_parameter("WqT", [T, F], FP, isOutput=False)
    WmT = nc.declare_dram_parameter("WmT", [F, C], FP, isOutput=False)
    brel = nc.declare_dram_parameter("brel", [F, 1], FP, isOutput=False)
    bmlp = nc.declare_dram_parameter("bmlp", [C, 1], FP, isOutput=False)
    I128 = nc.declare_dram_parameter("I128", [128, 128], FP, isOutput=False)
    Sout = nc.declare_dram_parameter("S_out", [ML, C], FP, isOutput=True)
    Lout = nc.declare_dram_parameter("losses", [1, 2], FP, isOutput=True)

    with tile.TileContext(nc) as tc:
        _body(nc, tc, Ash, Xf, Xloc, WrT, WqT, WmT, brel, bmlp, I128,
              Sout, Lout)

    nc.compile()
    return nc


def _body(nc, tc, Ash, Xf, Xloc, WrT, WqT, WmT, brel, bmlp, I128,
          Sout, Lout):
    fx = mybir.ActivationFunctionType
    alu = mybir.AluOpType

    ctx = ExitStack()
    sb = ctx.enter_context(tc.tile_pool(name="sb", bufs=1))
    dram = ctx.enter_context(tc.tile_pool(name="dram", bufs=1, space="DRAM"))

    # ---- persistent SBUF arrays ----
    # X tiles [128, 257]: col 256 is the ones column (gives deg for free)
    xn = [sb.tile([128, 257], FP, name=f"xn{k}", tag=f"xn{k}") for k in range(KT)]
    ident = sb.tile([128, 128], FP, name="ident", tag="ident")
    nc.sync.dma_start(ident[:, :], I128[:, :])
    wr = [sb.tile([128, F], FP, name=f"wr{t}", tag=f"wr{t}") for t in range(2)]
    wq = [sb.tile([128, F], FP, name=f"wq{t}", tag=f"wq{t}") for t in range(2)]
    wm = sb.tile([F, C], FP, name="wm", tag="wm")
    brel_sb = sb.tile([F, 1], FP, name="brel_sb", tag="brel_sb")
    bmlp_sb = sb.tile([C, 1], FP, name="bmlp_sb", tag="bmlp_sb")
    for t in range(2):
        nc.sync.dma_start(wr[t][:, :], WrT[t * 128:(t + 1) * 128, :])
        nc.sync.dma_start(wq[t][:, :], WqT[t * 128:(t + 1) * 128, :])
    nc.sync.dma_start(wm[:, :], WmT[:, :])
    nc.sync.dma_start(brel_sb[:, :], brel[:, :])
    nc.sync.dma_start(bmlp_sb[:, :], bmlp[:, :])

    # ---- load X, per-tile partial stats ----
    pstat = sb.tile([128, KT], FP, name="pstat", tag="pstat")
    pstat2 = sb.tile([128, KT], FP, name="pstat2", tag="pstat2")
    for k in range(KT):
        nc.sync.dma_start(xn[k][:, 0:T], Xf[k * 128:(k + 1) * 128, :])
        nc.gpsimd.memset(xn[k][:, T:T + 1], 1.0)
        nc.vector.tensor_reduce(pstat[:, k:k + 1], xn[k][:, 0:T],
                                mybir.AxisListType.X, alu.add)
        sq = sb.tile([128, T], FP, name="sq", tag="sq", bufs=2)
        nc.scalar.activation(sq[:, :], xn[k][:, 0:T], fx.Square,
                             accum_out=pstat2[:, k:k + 1])

    # ---- global mean/var -> r = rsqrt(var+eps), nmr = -mu*r  (per partition,
    # all 128 partitions hold identical values after partition_all_reduce) ----
    st2 = sb.tile([128, 2], FP, name="st2", tag="st2")
    nc.vector.tensor_reduce(st2[:, 0:1], pstat[:, :], mybir.AxisListType.X, alu.add)
    nc.vector.tensor_reduce(st2[:, 1:2], pstat2[:, :], mybir.AxisListType.X, alu.add)
    stall = sb.tile([128, 2], FP, name="stall", tag="stall")
    nc.gpsimd.partition_all_reduce(stall[:, :], st2[:, :], channels=128,
                                   reduce_op=bass_isa.ReduceOp.add)
    mu = sb.tile([128, 1], FP, name="mu", tag="mu")
    ex2 = sb.tile([128, 1], FP, name="ex2", tag="ex2")
    nc.scalar.mul(mu[:, :], stall[:, 0:1], 1.0 / (N * T))
    nc.scalar.mul(ex2[:, :], stall[:, 1:2], 1.0 / (N * T))
    mu2 = sb.tile([128, 1], FP, name="mu2", tag="mu2")
    nc.scalar.activation(mu2[:, :], mu[:, :], fx.Square)
    var = sb.tile([128, 1], FP, name="var", tag="var")
    nc.vector.tensor_sub(var[:, :], ex2[:, :], mu2[:, :])
    nc.vector.tensor_scalar_add(var[:, :], var[:, :], EPS)
    sd = sb.tile([128, 1], FP, name="sd", tag="sd")
    nc.scalar.activation(sd[:, :], var[:, :], fx.Sqrt)
    rr = sb.tile([128, 1], FP, name="rr", tag="rr")
    nc.vector.reciprocal(rr[:, :], sd[:, :])
    nmr = sb.tile([128, 1], FP, name="nmr", tag="nmr")
    nc.vector.tensor_mul(nmr[:, :], mu[:, :], rr[:, :])
    nc.vector.tensor_scalar_mul(nmr[:, :], nmr[:, :], -1.0)

    # ---- normalize all X tiles in place: xn = x*r - mu*r ----
    for k in range(KT):
        nc.vector.tensor_scalar(xn[k][:, 0:T], xn[k][:, 0:T],
                                rr[:, 0:1], nmr[:, 0:1], alu.mult, alu.add)

    # ---- local X rows (for the lin_root term), normalized ----
    xl = [sb.tile([128, T], FP, name=f"xl{m}", tag=f"xl{m}") for m in range(MT)]
    for m in range(MT):
        nc.sync.dma_start(xl[m][:, :], Xloc[m * 128:(m + 1) * 128, :])
        nc.vector.tensor_scalar(xl[m][:, :], xl[m][:, :],
                                rr[:, 0:1], nmr[:, 0:1], alu.mult, alu.add)

    # ---- phase 1: [agg | deg] = Ash^T @ [Xn | 1] ----
    agg = [sb.tile([128, T], FP, name=f"agg{m}", tag=f"agg{m}") for m in range(MT)]
    dinv = sb.tile([128, MT], FP, name="dinv", tag="dinv")
    with tc.tile_pool(name="psA", bufs=1, space="PSUM") as psA:
        ps1 = [psA.tile([128, T + 1], FP, name=f"ps1_{m}", tag=f"ps1_{m}")
               for m in range(MT)]
        with tc.tile_pool(name="apool", bufs=6) as apool:
            for k in range(KT):
                asl = apool.tile([128, ML], FP, name="asl", tag="asl")
                nc.sync.dma_start(asl[:, :], Ash[k * 128:(k + 1) * 128, :])
                for m in range(MT):
                    nc.tensor.matmul(ps1[m][:, :], asl[:, m * 128:(m + 1) * 128],
                                     xn[k][:, :], start=(k == 0),
                                     stop=(k == KT - 1))
            # evict: agg rows + dinv = 1/sqrt(deg)
            dsq = sb.tile([128, MT], FP, name="dsq", tag="dsq")
            for m in range(MT):
                nc.vector.tensor_copy(agg[m][:, :], ps1[m][:, 0:T])
                nc.scalar.activation(dsq[:, m:m + 1], ps1[m][:, T:T + 1], fx.Sqrt)
                nc.vector.reciprocal(dinv[:, m:m + 1], dsq[:, m:m + 1])

    # ---- transposes: aggT/xlT [T, ML], then H^T, S^T, S, softmax ----
    aggT = [sb.tile([128, ML], FP, name=f"aggT{t}", tag=f"aggT{t}") for t in range(2)]
    xlT = [sb.tile([128, ML], FP, name=f"xlT{t}", tag=f"xlT{t}") for t in range(2)]
    hT = sb.tile([128, ML], FP, name="hT", tag="hT")
    sT = sb.tile([C, ML], FP, name="sT", tag="sT")
    svals = [sb.tile([128, C], FP, name=f"svals{m}", tag=f"svals{m}")
             for m in range(MT)]
    ssm = [sb.tile([128, C], FP, name=f"ssm{m}", tag=f"ssm{m}")
           for m in range(MT)]
    with tc.tile_pool(name="psB", bufs=1, space="PSUM") as psB:
        for t in range(2):
            for m in range(MT):
                tp = psB.tile([128, 128], FP, name="tp", tag="tp", bufs=2)
                nc.tensor.transpose(tp[:, :], agg[m][:, t * 128:(t + 1) * 128],
                                    ident[:, :])
                nc.vector.tensor_copy(aggT[t][:, m * 128:(m + 1) * 128], tp[:, :])
                tp2 = psB.tile([128, 128], FP, name="tp2", tag="tp2", bufs=2)
                nc.tensor.transpose(tp2[:, :], xl[m][:, t * 128:(t + 1) * 128],
                                    ident[:, :])
                nc.vector.tensor_copy(xlT[t][:, m * 128:(m + 1) * 128], tp2[:, :])
        # H^T = WrT.T @ aggT + WqT.T @ xlT + b_rel
        for ch in range(2):
            hps = psB.tile([128, 512], FP, name="hps", tag="hps", bufs=1)
            for t in range(2):
                nc.tensor.matmul(hps[:, :], wr[t][:, :],
                                 aggT[t][:, ch * 512:(ch + 1) * 512],
                                 start=(t == 0), stop=False)
            for t in range(2):
                nc.tensor.matmul(hps[:, :], wq[t][:, :],
                                 xlT[t][:, ch * 512:(ch + 1) * 512],
                                 start=False, stop=(t == 1))
            nc.scalar.activation(hT[:, ch * 512:(ch + 1) * 512], hps[:, :],
                                 fx.Identity, bias=brel_sb[:, 0:1])
        # S^T = WmT.T @ H^T + b_mlp
        for ch in range(2):
            sps = psB.tile([C, 512], FP, name="sps", tag="sps", bufs=1)
            nc.tensor.matmul(sps[:, :], wm[:, :],
                             hT[:, ch * 512:(ch + 1) * 512],
                             start=True, stop=True)
            nc.scalar.activation(sT[:, ch * 512:(ch + 1) * 512], sps[:, :],
                                 fx.Identity, bias=bmlp_sb[:, 0:1])
        # S tiles (nodes on partitions) + output + softmax
        for m in range(MT):
            stp = psB.tile([128, C], FP, name="stp", tag="stp", bufs=2)
            nc.tensor.transpose(stp[:, :], sT[:, m * 128:(m + 1) * 128],
                                ident[0:C, 0:C])
            nc.vector.tensor_copy(svals[m][:, :], stp[:, :])
            nc.sync.dma_start(Sout[m * 128:(m + 1) * 128, :], svals[m][:, :])
            nmx = sb.tile([128, 1], FP, name="nmx", tag="nmx", bufs=2)
            nc.vector.tensor_reduce(nmx[:, :], svals[m][:, :],
                                    mybir.AxisListType.X, alu.max, negate=True)
            esum = sb.tile([128, 1], FP, name="esum", tag="esum", bufs=2)
            nc.scalar.activation(ssm[m][:, :], svals[m][:, :], fx.Exp,
                                 bias=nmx[:, 0:1], accum_out=esum[:, :])
            rsum = sb.tile([128, 1], FP, name="rsum", tag="rsum", bufs=2)
            nc.vector.reciprocal(rsum[:, :], esum[:, :])
            nc.vector.tensor_scalar_mul(ssm[m][:, :], ssm[m][:, :], rsum[:, 0:1])

    # ---- all-gather Z = [dinv*s | dinv] ----
    cc_in1 = dram.tile([ML, C + 1], FP, name="cc_in1", tag="cc_in1")
    cc_out1 = dram.tile([N, C + 1], FP, name="cc_out1", tag="cc_out1",
                        addr_space="Shared")
    for m in range(MT):
        z = sb.tile([128, C + 1], FP, name="z", tag="z", bufs=4)
        nc.vector.tensor_scalar_mul(z[:, 0:C], ssm[m][:, :], dinv[:, m:m + 1])
        nc.vector.tensor_copy(z[:, C:C + 1], dinv[:, m:m + 1])
        nc.sync.dma_start(cc_in1[m * 128:(m + 1) * 128, :], z[:, :])
    nc.gpsimd.collective_compute(
        "AllGather", alu.bypass,
        replica_groups=[list(range(NCORES))],
        ins=[cc_in1.opt()], outs=[cc_out1.opt()],
    )
    zt = [sb.tile([128, C + 1], FP, name=f"zt{k}", tag=f"zt{k}")
          for k in range(KT)]
    for k in range(KT):
        nc.sync.dma_start(zt[k][:, :], cc_out1[k * 128:(k + 1) * 128, :])

    # ---- phase 2: [U | Ad] = Ash^T @ [dinv*s | dinv] ----
    # as_m layout [128, 97]: 0:32 As=dinv*U, 32 d=dinv*Ad, 33:65 s, 65:97 d*s
    asx = [sb.tile([128, 3 * C + 1], FP, name=f"asx{m}", tag=f"asx{m}")
           for m in range(MT)]
    with tc.tile_pool(name="psC", bufs=1, space="PSUM") as psC:
        ps2 = [psC.tile([128, C + 1], FP, name=f"ps2_{m}", tag=f"ps2_{m}")
               for m in range(MT)]
        with tc.tile_pool(name="apool2", bufs=6) as apool2:
            for k in range(KT):
                asl2 = apool2.tile([128, ML], FP, name="asl2", tag="asl2")
                nc.sync.dma_start(asl2[:, :], Ash[k * 128:(k + 1) * 128, :])
                for m in range(MT):
                    nc.tensor.matmul(ps2[m][:, :],
                                     asl2[:, m * 128:(m + 1) * 128],
                                     zt[k][:, :], start=(k == 0),
                                     stop=(k == KT - 1))
            for m in range(MT):
                nc.vector.tensor_scalar_mul(asx[m][:, 0:C + 1], ps2[m][:, :],
                                            dinv[:, m:m + 1])
                nc.vector.tensor_copy(asx[m][:, C + 1:2 * C + 1], ssm[m][:, :])
                nc.vector.tensor_scalar_mul(asx[m][:, 2 * C + 1:3 * C + 1],
                                            ssm[m][:, :], asx[m][:, C:C + 1])

    # ---- local partials P = s^T @ [As | d | s | d*s]  -> pack [32, 65] ----
    i32 = sb.tile([C, C], FP, name="i32", tag="i32")
    nc.vector.tensor_copy(i32[:, :], ident[0:C, 0:C])
    pack = sb.tile([C, 2 * C + 1], FP, name="pack", tag="pack")
    with tc.tile_pool(name="psD", bufs=1, space="PSUM") as psD:
        pp = psD.tile([C, 3 * C + 1], FP, name="pp", tag="pp")
        for m in range(MT):
            nc.tensor.matmul(pp[:, :], ssm[m][:, :], asx[m][:, :],
                             start=(m == 0), stop=(m == MT - 1))
        nc.vector.tensor_copy(pack[:, 0:C], pp[:, 0:C])
        nc.vector.tensor_copy(pack[:, C:2 * C], pp[:, C + 1:2 * C + 1])
        # den partial = trace(s^T (d*s))
        tds = sb.tile([C, C], FP, name="tds", tag="tds")
        nc.vector.tensor_copy(tds[:, :], pp[:, 2 * C + 1:3 * C + 1])
        nc.vector.tensor_mul(tds[:, :], tds[:, :], i32[:, :])
        denp = sb.tile([C, 1], FP, name="denp", tag="denp")
        nc.vector.tensor_reduce(denp[:, :], tds[:, :], mybir.AxisListType.X,
                                alu.add)
        denall = sb.tile([C, 1], FP, name="denall", tag="denall")
        nc.gpsimd.partition_all_reduce(denall[:, :], denp[:, :], channels=C,
                                       reduce_op=bass_isa.ReduceOp.add)
        nc.vector.tensor_copy(pack[:, 2 * C:2 * C + 1], denall[:, :])

    # ---- all-reduce the [32, 65] partials ----
    cc_in2 = dram.tile([C, 2 * C + 1], FP, name="cc_in2", tag="cc_in2")
    cc_out2 = dram.tile([C, 2 * C + 1], FP, name="cc_out2", tag="cc_out2",
                        addr_space="Shared")
    nc.sync.dma_start(cc_in2[:, :], pack[:, :])
    nc.gpsimd.collective_compute(
        "AllReduce", alu.add,
        replica_groups=[list(range(NCORES))],
        ins=[cc_in2.opt()], outs=[cc_out2.opt()],
    )
    red = sb.tile([C, 2 * C + 1], FP, name="red", tag="red")
    nc.sync.dma_start(red[:, :], cc_out2[:, :])

    # ---- losses (computed redundantly on 32 partitions) ----
    # mincut: num = trace(adj); loss_mc = -(num/den)
    t1 = sb.tile([C, C], FP, name="t1", tag="t1")
    nc.vector.tensor_mul(t1[:, :], red[:, 0:C], i32[:, :])
    diag = sb.tile([C, 1], FP, name="diag", tag="diag")
    nc.vector.tensor_reduce(diag[:, :], t1[:, :], mybir.AxisListType.X, alu.add)
    num32 = sb.tile([C, 1], FP, name="num32", tag="num32")
    nc.gpsimd.partition_all_reduce(num32[:, :], diag[:, :], channels=C,
                                   reduce_op=bass_isa.ReduceOp.add)
    recden = sb.tile([C, 1], FP, name="recden", tag="recden")
    nc.vector.reciprocal(recden[:, :], red[:, 2 * C:2 * C + 1])
    lm = sb.tile([C, 1], FP, name="lm", tag="lm")
    nc.vector.tensor_mul(lm[:, :], num32[:, :], recden[:, :])
    nc.vector.tensor_scalar_mul(lm[:, :], lm[:, :], -1.0)
    # ortho: loss_o = || ss/||ss||_F - I/sqrt(C) ||_F
    ssq = sb.tile([C, C], FP, name="ssq", tag="ssq")
    rowsq = sb.tile([C, 1], FP, name="rowsq", tag="rowsq")
    nc.scalar.activation(ssq[:, :], red[:, C:2 * C], fx.Square,
                         accum_out=rowsq[:, :])
    fro2 = sb.tile([C, 1], FP, name="fro2", tag="fro2")
    nc.gpsimd.partition_all_reduce(fro2[:, :], rowsq[:, :], channels=C,
                                   reduce_op=bass_isa.ReduceOp.add)
    fro = sb.tile([C, 1], FP, name="fro", tag="fro")
    nc.scalar.activation(fro[:, :], fro2[:, :], fx.Sqrt)
    rf = sb.tile([C, 1], FP, name="rf", tag="rf")
    nc.vector.reciprocal(rf[:, :], fro[:, :])
    en = sb.tile([C, C], FP, name="en", tag="en")
    nc.vector.tensor_scalar_mul(en[:, :], red[:, C:2 * C], rf[:, 0:1])
    i2 = sb.tile([C, C], FP, name="i2", tag="i2")
    nc.scalar.mul(i2[:, :], i32[:, :], 1.0 / float(np.sqrt(C)))
    nc.vector.tensor_sub(en[:, :], en[:, :], i2[:, :])
    e2 = sb.tile([C, C], FP, name="e2", tag="e2")
    e2r = sb.tile([C, 1], FP, name="e2r", tag="e2r")
    nc.scalar.activation(e2[:, :], en[:, :], fx.Square, accum_out=e2r[:, :])
    lo2 = sb.tile([C, 1], FP, name="lo2", tag="lo2")
    nc.gpsimd.partition_all_reduce(lo2[:, :], e2r[:, :], channels=C,
                                   reduce_op=bass_isa.ReduceOp.add)
    lo = sb.tile([C, 1], FP, name="lo", tag="lo")
    nc.scalar.activation(lo[:, :], lo2[:, :], fx.Sqrt)

    lout = sb.tile([1, 2], FP, name="lout", tag="lout")
    nc.vector.tensor_copy(lout[0:1, 0:1], lm[0:1, 0:1])
    nc.vector.tensor_copy(lout[0:1, 1:2], lo[0:1, 0:1])
    nc.sync.dma_start(Lout[:, :], lout[:, :])

    ctx.close()


def _prep_inputs(X, A, W_rel, b_rel, W_root, W_mlp, b_mlp):
    X = np.ascontiguousarray(np.asarray(X, dtype=np.float32))
    A = np.asarray(A, dtype=np.float32)
    WrT = np.ascontiguousarray(np.asarray(W_rel, np.float32).T)     # [T, F]
    WqT = np.ascontiguousarray(np.asarray(W_root, np.float32).T)    # [T, F]
    WmT = np.ascontiguousarray(np.asarray(W_mlp, np.float32).T)     # [F, C]
    brel = np.ascontiguousarray(np.asarray(b_rel, np.float32).reshape(F, 1))
    bmlp = np.ascontiguousarray(np.asarray(b_mlp, np.float32).reshape(C, 1))
    I128_np = np.eye(128, dtype=np.float32)
    in_maps = []
    for c in range(NCORES):
        cols = slice(c * ML, (c + 1) * ML)
        in_maps.append({
            "Ash": np.ascontiguousarray(A[:, cols]),
            "Xf": X,
            "Xloc": np.ascontiguousarray(X[cols, :]),
            "WrT": WrT, "WqT": WqT, "WmT": WmT,
            "brel": brel, "bmlp": bmlp, "I128": I128_np,
        })
    return in_maps


def kernel(X, A, W_rel, b_rel, W_root, W_mlp, b_mlp):
    global last_exec_time_ns
    if "nc" not in _cache:
        _cache["nc"] = _build()
    nc = _cache["nc"]
    in_maps = _prep_inputs(X, A, W_rel, b_rel, W_root, W_mlp, b_mlp)
    trace = os.environ.get("BENCH_TRACE", "0") == "1"
    res = run_bass_kernel_spmd(nc, in_maps, list(range(NCORES)), trace=trace)
    last_exec_time_ns = res.exec_time_ns
    S = np.concatenate([res.results[c]["S_out"] for c in range(NCORES)], axis=0)
    losses = res.results[0]["losses"]
    return (S.astype(np.float32),
            np.float32(losses[0, 0]),
            np.float32(losses[0, 1]))


# revision 8
# speedup vs baseline: 1201.9904x; 1201.9904x over previous
"""Trainium2 Bass kernel for GNN message passing + dense_mincut_pool losses.

Computes, given X [8192,256], A [8192,8192] (symmetric 0/1 with self loops)
and GraphConv/MLP weights:
    S = GraphConv(LayerNorm(X), A-pattern) @ mlp    -> [8192, 32]
    loss_mc, loss_o  (dense_mincut_pool losses on softmax(S))

Distribution: A is sharded column-wise across 8 NeuronCores (core k gets
A[:, k*1024:(k+1)*1024], which by symmetry equals A[rows_k, :]^T). X is
replicated. Each core computes S for its 1024 nodes; softmaxed cluster
assignments are all-gathered for the s^T A s contraction; tiny [C,C]
partials are all-reduced; losses are computed redundantly on every core.
"""

import os
import sys

sys.path.insert(0, "/opt/trn_rl_repo")
os.environ.setdefault("MYCRO_LOCAL_CACHE", "1")

import numpy as np
from contextlib import ExitStack  # noqa: F401

import concourse.bass as bass
import concourse.bacc as bacc
import concourse.mybir as mybir
import concourse.tile as tile
import concourse.bass_isa as bass_isa
from concourse.bass_utils import run_bass_kernel_spmd

N, T, F, C = 8192, 256, 128, 32
NCORES = 8
ML = N // NCORES       # 1024 local nodes per core
KT = N // 128          # 64 contraction tiles
MT = ML // 128         # 8 local node tiles
EPS = 1e-5
FP = mybir.dt.float32

_cache = {}
last_exec_time_ns = None


def _build(reps=1):
    nc = bacc.Bacc("TRN2", target_bir_lowering=False, debug=False,
                   num_devices=NCORES)

    Ash = nc.declare_dram_parameter("Ash", [N, ML], FP, isOutput=False)
    Xf = nc.declare_dram_parameter("Xf", [N, T], FP, isOutput=False)
    Xloc = nc.declare_dram_parameter("Xloc", [ML, T], FP, isOutput=False)
    WrT = nc.declare_dram_parameter("WrT", [T, F], FP, isOutput=False)
    WqT = nc.declare_dram_parameter("WqT", [T, F], FP, isOutput=False)
    WmT = nc.declare_dram_parameter("WmT", [F, C], FP, isOutput=False)
    brel = nc.declare_dram_parameter("brel", [F, 1], FP, isOutput=False)
    bmlp = nc.declare_dram_parameter("bmlp", [C, 1], FP, isOutput=False)
    I128 = nc.declare_dram_parameter("I128", [128, 128], FP, isOutput=False)
    Sout = nc.declare_dram_parameter("S_out", [ML, C], FP, isOutput=True)
    Lout = nc.declare_dram_parameter("losses", [1, 2], FP, isOutput=True)

    with tile.TileContext(nc) as tc:
        for rep in range(reps):
            _body(nc, tc, rep, Ash, Xf, Xloc, WrT, WqT, WmT, brel, bmlp,
                  I128, Sout, Lout)

    nc.compile()
    return nc


def _body(nc, tc, rep, Ash, Xf, Xloc, WrT, WqT, WmT, brel, bmlp, I128,
          Sout, Lout):
    fx = mybir.ActivationFunctionType
    alu = mybir.AluOpType

    ctx = ExitStack()
    sb = ctx.enter_context(tc.tile_pool(name=f"sb{rep}", bufs=1))
    dram = ctx.enter_context(tc.tile_pool(name=f"dram{rep}", bufs=1,
                                          space="DRAM"))

    # ---- persistent SBUF arrays ----
    # X tiles [128, 257]: col 256 is the ones column (gives deg for free)
    xn = [sb.tile([128, 257], FP, name=f"xn{k}", tag=f"xn{k}") for k in range(KT)]
    ident = sb.tile([128, 128], FP, name="ident", tag="ident")
    nc.sync.dma_start(ident[:, :], I128[:, :])
    wr = [sb.tile([128, F], FP, name=f"wr{t}", tag=f"wr{t}") for t in range(2)]
    wq = [sb.tile([128, F], FP, name=f"wq{t}", tag=f"wq{t}") for t in range(2)]
    wm = sb.tile([F, C], FP, name="wm", tag="wm")
    brel_sb = sb.tile([F, 1], FP, name="brel_sb", tag="brel_sb")
    bmlp_sb = sb.tile([C, 1], FP, name="bmlp_sb", tag="bmlp_sb")
    for t in range(2):
        nc.sync.dma_start(wr[t][:, :], WrT[t * 128:(t + 1) * 128, :])
        nc.sync.dma_start(wq[t][:, :], WqT[t * 128:(t + 1) * 128, :])
    nc.sync.dma_start(wm[:, :], WmT[:, :])
    nc.sync.dma_start(brel_sb[:, :], brel[:, :])
    nc.sync.dma_start(bmlp_sb[:, :], bmlp[:, :])

    # ---- load X, per-tile partial stats ----
    pstat = sb.tile([128, KT], FP, name="pstat", tag="pstat")
    pstat2 = sb.tile([128, KT], FP, name="pstat2", tag="pstat2")
    for k in range(KT):
        nc.sync.dma_start(xn[k][:, 0:T], Xf[k * 128:(k + 1) * 128, :])
        nc.gpsimd.memset(xn[k][:, T:T + 1], 1.0)
        nc.vector.tensor_reduce(pstat[:, k:k + 1], xn[k][:, 0:T],
                                mybir.AxisListType.X, alu.add)
        sq = sb.tile([128, T], FP, name="sq", tag="sq", bufs=2)
        nc.scalar.activation(sq[:, :], xn[k][:, 0:T], fx.Square,
                             accum_out=pstat2[:, k:k + 1])

    # ---- global mean/var -> r = rsqrt(var+eps), nmr = -mu*r  (per partition,
    # all 128 partitions hold identical values after partition_all_reduce) ----
    st2 = sb.tile([128, 2], FP, name="st2", tag="st2")
    nc.vector.tensor_reduce(st2[:, 0:1], pstat[:, :], mybir.AxisListType.X, alu.add)
    nc.vector.tensor_reduce(st2[:, 1:2], pstat2[:, :], mybir.AxisListType.X, alu.add)
    stall = sb.tile([128, 2], FP, name="stall", tag="stall")
    nc.gpsimd.partition_all_reduce(stall[:, :], st2[:, :], channels=128,
                                   reduce_op=bass_isa.ReduceOp.add)
    mu = sb.tile([128, 1], FP, name="mu", tag="mu")
    ex2 = sb.tile([128, 1], FP, name="ex2", tag="ex2")
    nc.scalar.mul(mu[:, :], stall[:, 0:1], 1.0 / (N * T))
    nc.scalar.mul(ex2[:, :], stall[:, 1:2], 1.0 / (N * T))
    mu2 = sb.tile([128, 1], FP, name="mu2", tag="mu2")
    nc.scalar.activation(mu2[:, :], mu[:, :], fx.Square)
    var = sb.tile([128, 1], FP, name="var", tag="var")
    nc.vector.tensor_sub(var[:, :], ex2[:, :], mu2[:, :])
    nc.vector.tensor_scalar_add(var[:, :], var[:, :], EPS)
    sd = sb.tile([128, 1], FP, name="sd", tag="sd")
    nc.scalar.activation(sd[:, :], var[:, :], fx.Sqrt)
    rr = sb.tile([128, 1], FP, name="rr", tag="rr")
    nc.vector.reciprocal(rr[:, :], sd[:, :])
    nmr = sb.tile([128, 1], FP, name="nmr", tag="nmr")
    nc.vector.tensor_mul(nmr[:, :], mu[:, :], rr[:, :])
    nc.vector.tensor_scalar_mul(nmr[:, :], nmr[:, :], -1.0)

    # ---- normalize all X tiles in place: xn = x*r - mu*r ----
    for k in range(KT):
        nc.vector.tensor_scalar(xn[k][:, 0:T], xn[k][:, 0:T],
                                rr[:, 0:1], nmr[:, 0:1], alu.mult, alu.add)

    # ---- local X rows (for the lin_root term), normalized ----
    xl = [sb.tile([128, T], FP, name=f"xl{m}", tag=f"xl{m}") for m in range(MT)]
    for m in range(MT):
        nc.sync.dma_start(xl[m][:, :], Xloc[m * 128:(m + 1) * 128, :])
        nc.vector.tensor_scalar(xl[m][:, :], xl[m][:, :],
                                rr[:, 0:1], nmr[:, 0:1], alu.mult, alu.add)

    # ---- phase 1: [agg | deg] = Ash^T @ [Xn | 1] ----
    agg = [sb.tile([128, T], FP, name=f"agg{m}", tag=f"agg{m}") for m in range(MT)]
    dinv = sb.tile([128, MT], FP, name="dinv", tag="dinv")
    with tc.tile_pool(name=f"psA{rep}", bufs=1, space="PSUM") as psA:
        ps1 = [psA.tile([128, T + 1], FP, name=f"ps1_{m}", tag=f"ps1_{m}")
               for m in range(MT)]
        with tc.tile_pool(name=f"apool{rep}", bufs=6) as apool:
            for k in range(KT):
                asl = apool.tile([128, ML], FP, name="asl", tag="asl")
                nc.sync.dma_start(asl[:, :], Ash[k * 128:(k + 1) * 128, :])
                for m in range(MT):
                    nc.tensor.matmul(ps1[m][:, :], asl[:, m * 128:(m + 1) * 128],
                                     xn[k][:, :], start=(k == 0),
                                     stop=(k == KT - 1))
            # evict: agg rows + dinv = 1/sqrt(deg)
            dsq = sb.tile([128, MT], FP, name="dsq", tag="dsq")
            for m in range(MT):
                nc.vector.tensor_copy(agg[m][:, :], ps1[m][:, 0:T])
                nc.scalar.activation(dsq[:, m:m + 1], ps1[m][:, T:T + 1], fx.Sqrt)
                nc.vector.reciprocal(dinv[:, m:m + 1], dsq[:, m:m + 1])

    # ---- transposes: aggT/xlT [T, ML], then H^T, S^T, S, softmax ----
    aggT = [sb.tile([128, ML], FP, name=f"aggT{t}", tag=f"aggT{t}") for t in range(2)]
    xlT = [sb.tile([128, ML], FP, name=f"xlT{t}", tag=f"xlT{t}") for t in range(2)]
    hT = sb.tile([128, ML], FP, name="hT", tag="hT")
    sT = sb.tile([C, ML], FP, name="sT", tag="sT")
    svals = [sb.tile([128, C], FP, name=f"svals{m}", tag=f"svals{m}")
             for m in range(MT)]
    ssm = [sb.tile([128, C], FP, name=f"ssm{m}", tag=f"ssm{m}")
           for m in range(MT)]
    with tc.tile_pool(name=f"psB{rep}", bufs=1, space="PSUM") as psB:
        for t in range(2):
            for m in range(MT):
                tp = psB.tile([128, 128], FP, name="tp", tag="tp", bufs=2)
                nc.tensor.transpose(tp[:, :], agg[m][:, t * 128:(t + 1) * 128],
                                    ident[:, :])
                nc.vector.tensor_copy(aggT[t][:, m * 128:(m + 1) * 128], tp[:, :])
                tp2 = psB.tile([128, 128], FP, name="tp2", tag="tp2", bufs=2)
                nc.tensor.transpose(tp2[:, :], xl[m][:, t * 128:(t + 1) * 128],
                                    ident[:, :])
                nc.vector.tensor_copy(xlT[t][:, m * 128:(m + 1) * 128], tp2[:, :])
        # H^T = WrT.T @ aggT + WqT.T @ xlT + b_rel
        for ch in range(2):
            hps = psB.tile([128, 512], FP, name="hps", tag="hps", bufs=1)
            for t in range(2):
                nc.tensor.matmul(hps[:, :], wr[t][:, :],
                                 aggT[t][:, ch * 512:(ch + 1) * 512],
                                 start=(t == 0), stop=False)
            for t in range(2):
                nc.tensor.matmul(hps[:, :], wq[t][:, :],
                                 xlT[t][:, ch * 512:(ch + 1) * 512],
                                 start=False, stop=(t == 1))
            nc.scalar.activation(hT[:, ch * 512:(ch + 1) * 512], hps[:, :],
                                 fx.Identity, bias=brel_sb[:, 0:1])
        # S^T = WmT.T @ H^T + b_mlp
        for ch in range(2):
            sps = psB.tile([C, 512], FP, name="sps", tag="sps", bufs=1)
            nc.tensor.matmul(sps[:, :], wm[:, :],
                             hT[:, ch * 512:(ch + 1) * 512],
                             start=True, stop=True)
            nc.scalar.activation(sT[:, ch * 512:(ch + 1) * 512], sps[:, :],
                                 fx.Identity, bias=bmlp_sb[:, 0:1])
        # S tiles (nodes on partitions) + output + softmax
        for m in range(MT):
            stp = psB.tile([128, C], FP, name="stp", tag="stp", bufs=2)
            nc.tensor.transpose(stp[:, :], sT[:, m * 128:(m + 1) * 128],
                                ident[0:C, 0:C])
            nc.vector.tensor_copy(svals[m][:, :], stp[:, :])
            nc.sync.dma_start(Sout[m * 128:(m + 1) * 128, :], svals[m][:, :])
            nmx = sb.tile([128, 1], FP, name="nmx", tag="nmx", bufs=2)
            nc.vector.tensor_reduce(nmx[:, :], svals[m][:, :],
                                    mybir.AxisListType.X, alu.max, negate=True)
            esum = sb.tile([128, 1], FP, name="esum", tag="esum", bufs=2)
            nc.scalar.activation(ssm[m][:, :], svals[m][:, :], fx.Exp,
                                 bias=nmx[:, 0:1], accum_out=esum[:, :])
            rsum = sb.tile([128, 1], FP, name="rsum", tag="rsum", bufs=2)
            nc.vector.reciprocal(rsum[:, :], esum[:, :])
            nc.vector.tensor_scalar_mul(ssm[m][:, :], ssm[m][:, :], rsum[:, 0:1])

    # ---- all-gather Z = [dinv*s | dinv] ----
    cc_in1 = dram.tile([ML, C + 1], FP, name="cc_in1", tag="cc_in1")
    cc_out1 = dram.tile([N, C + 1], FP, name="cc_out1", tag="cc_out1",
                        addr_space="Shared")
    for m in range(MT):
        z = sb.tile([128, C + 1], FP, name="z", tag="z", bufs=4)
        nc.vector.tensor_scalar_mul(z[:, 0:C], ssm[m][:, :], dinv[:, m:m + 1])
        nc.vector.tensor_copy(z[:, C:C + 1], dinv[:, m:m + 1])
        nc.sync.dma_start(cc_in1[m * 128:(m + 1) * 128, :], z[:, :])
    nc.gpsimd.collective_compute(
        "AllGather", alu.bypass,
        replica_groups=[list(range(NCORES))],
        ins=[cc_in1.opt()], outs=[cc_out1.opt()],
    )
    zt = [sb.tile([128, C + 1], FP, name=f"zt{k}", tag=f"zt{k}")
          for k in range(KT)]
    for k in range(KT):
        nc.sync.dma_start(zt[k][:, :], cc_out1[k * 128:(k + 1) * 128, :])

    # ---- phase 2: [U | Ad] = Ash^T @ [dinv*s | dinv] ----
    # as_m layout [128, 97]: 0:32 As=dinv*U, 32 d=dinv*Ad, 33:65 s, 65:97 d*s
    asx = [sb.tile([128, 3 * C + 1], FP, name=f"asx{m}", tag=f"asx{m}")
           for m in range(MT)]
    with tc.tile_pool(name=f"psC{rep}", bufs=1, space="PSUM") as psC:
        ps2 = [psC.tile([128, C + 1], FP, name=f"ps2_{m}", tag=f"ps2_{m}")
               for m in range(MT)]
        with tc.tile_pool(name=f"apool2{rep}", bufs=6) as apool2:
            for k in range(KT):
                asl2 = apool2.tile([128, ML], FP, name="asl2", tag="asl2")
                nc.sync.dma_start(asl2[:, :], Ash[k * 128:(k + 1) * 128, :])
                for m in range(MT):
                    nc.tensor.matmul(ps2[m][:, :],
                                     asl2[:, m * 128:(m + 1) * 128],
                                     zt[k][:, :], start=(k == 0),
                                     stop=(k == KT - 1))
            for m in range(MT):
                nc.vector.tensor_scalar_mul(asx[m][:, 0:C + 1], ps2[m][:, :],
                                            dinv[:, m:m + 1])
                nc.vector.tensor_copy(asx[m][:, C + 1:2 * C + 1], ssm[m][:, :])
                nc.vector.tensor_scalar_mul(asx[m][:, 2 * C + 1:3 * C + 1],
                                            ssm[m][:, :], asx[m][:, C:C + 1])

    # ---- local partials P = s^T @ [As | d | s | d*s]  -> pack [32, 65] ----
    i32 = sb.tile([C, C], FP, name="i32", tag="i32")
    nc.vector.tensor_copy(i32[:, :], ident[0:C, 0:C])
    pack = sb.tile([C, 2 * C + 1], FP, name="pack", tag="pack")
    with tc.tile_pool(name=f"psD{rep}", bufs=1, space="PSUM") as psD:
        pp = psD.tile([C, 3 * C + 1], FP, name="pp", tag="pp")
        for m in range(MT):
            nc.tensor.matmul(pp[:, :], ssm[m][:, :], asx[m][:, :],
                             start=(m == 0), stop=(m == MT - 1))
        nc.vector.tensor_copy(pack[:, 0:C], pp[:, 0:C])
        nc.vector.tensor_copy(pack[:, C:2 * C], pp[:, C + 1:2 * C + 1])
        # den partial = trace(s^T (d*s))
        tds = sb.tile([C, C], FP, name="tds", tag="tds")
        nc.vector.tensor_copy(tds[:, :], pp[:, 2 * C + 1:3 * C + 1])
        nc.vector.tensor_mul(tds[:, :], tds[:, :], i32[:, :])
        denp = sb.tile([C, 1], FP, name="denp", tag="denp")
        nc.vector.tensor_reduce(denp[:, :], tds[:, :], mybir.AxisListType.X,
                                alu.add)
        denall = sb.tile([C, 1], FP, name="denall", tag="denall")
        nc.gpsimd.partition_all_reduce(denall[:, :], denp[:, :], channels=C,
                                       reduce_op=bass_isa.ReduceOp.add)
        nc.vector.tensor_copy(pack[:, 2 * C:2 * C + 1], denall[:, :])

    # ---- all-reduce the [32, 65] partials ----
    cc_in2 = dram.tile([C, 2 * C + 1], FP, name="cc_in2", tag="cc_in2")
    cc_out2 = dram.tile([C, 2 * C + 1], FP, name="cc_out2", tag="cc_out2",
                        addr_space="Shared")
    nc.sync.dma_start(cc_in2[:, :], pack[:, :])
    nc.gpsimd.collective_compute(
        "AllReduce", alu.add,
        replica_groups=[list(range(NCORES))],
        ins=[cc_in2.opt()], outs=[cc_out2.opt()],
    )
    red = sb.tile([C, 2 * C + 1], FP, name="red", tag="red")
    nc.sync.dma_start(red[:, :], cc_out2[:, :])

    # ---- losses (computed redundantly on 32 partitions) ----
    # mincut: num = trace(adj); loss_mc = -(num/den)
    t1 = sb.tile([C, C], FP, name="t1", tag="t1")
    nc.vector.tensor_mul(t1[:, :], red[:, 0:C], i32[:, :])
    diag = sb.tile([C, 1], FP, name="diag", tag="diag")
    nc.vector.tensor_reduce(diag[:, :], t1[:, :], mybir.AxisListType.X, alu.add)
    num32 = sb.tile([C, 1], FP, name="num32", tag="num32")
    nc.gpsimd.partition_all_reduce(num32[:, :], diag[:, :], channels=C,
                                   reduce_op=bass_isa.ReduceOp.add)
    recden = sb.tile([C, 1], FP, name="recden", tag="recden")
    nc.vector.reciprocal(recden[:, :], red[:, 2 * C:2 * C + 1])
    lm = sb.tile([C, 1], FP, name="lm", tag="lm")
    nc.vector.tensor_mul(lm[:, :], num32[:, :], recden[:, :])
    nc.vector.tensor_scalar_mul(lm[:, :], lm[:, :], -1.0)
    # ortho: loss_o = || ss/||ss||_F - I/sqrt(C) ||_F
    ssq = sb.tile([C, C], FP, name="ssq", tag="ssq")
    rowsq = sb.tile([C, 1], FP, name="rowsq", tag="rowsq")
    nc.scalar.activation(ssq[:, :], red[:, C:2 * C], fx.Square,
                         accum_out=rowsq[:, :])
    fro2 = sb.tile([C, 1], FP, name="fro2", tag="fro2")
    nc.gpsimd.partition_all_reduce(fro2[:, :], rowsq[:, :], channels=C,
                                   reduce_op=bass_isa.ReduceOp.add)
    fro = sb.tile([C, 1], FP, name="fro", tag="fro")
    nc.scalar.activation(fro[:, :], fro2[:, :], fx.Sqrt)
    rf = sb.tile([C, 1], FP, name="rf", tag="rf")
    nc.vector.reciprocal(rf[:, :], fro[:, :])
    en = sb.tile([C, C], FP, name="en", tag="en")
    nc.vector.tensor_scalar_mul(en[:, :], red[:, C:2 * C], rf[:, 0:1])
    i2 = sb.tile([C, C], FP, name="i2", tag="i2")
    nc.scalar.mul(i2[:, :], i32[:, :], 1.0 / float(np.sqrt(C)))
    nc.vector.tensor_sub(en[:, :], en[:, :], i2[:, :])
    e2 = sb.tile([C, C], FP, name="e2", tag="e2")
    e2r = sb.tile([C, 1], FP, name="e2r", tag="e2r")
    nc.scalar.activation(e2[:, :], en[:, :], fx.Square, accum_out=e2r[:, :])
    lo2 = sb.tile([C, 1], FP, name="lo2", tag="lo2")
    nc.gpsimd.partition_all_reduce(lo2[:, :], e2r[:, :], channels=C,
                                   reduce_op=bass_isa.ReduceOp.add)
    lo = sb.tile([C, 1], FP, name="lo", tag="lo")
    nc.scalar.activation(lo[:, :], lo2[:, :], fx.Sqrt)

    lout = sb.tile([1, 2], FP, name="lout", tag="lout")
    nc.vector.tensor_copy(lout[0:1, 0:1], lm[0:1, 0:1])
    nc.vector.tensor_copy(lout[0:1, 1:2], lo[0:1, 0:1])
    nc.sync.dma_start(Lout[:, :], lout[:, :])

    ctx.close()


def _prep_inputs(X, A, W_rel, b_rel, W_root, W_mlp, b_mlp):
    X = np.ascontiguousarray(np.asarray(X, dtype=np.float32))
    A = np.asarray(A, dtype=np.float32)
    WrT = np.ascontiguousarray(np.asarray(W_rel, np.float32).T)     # [T, F]
    WqT = np.ascontiguousarray(np.asarray(W_root, np.float32).T)    # [T, F]
    WmT = np.ascontiguousarray(np.asarray(W_mlp, np.float32).T)     # [F, C]
    brel = np.ascontiguousarray(np.asarray(b_rel, np.float32).reshape(F, 1))
    bmlp = np.ascontiguousarray(np.asarray(b_mlp, np.float32).reshape(C, 1))
    I128_np = np.eye(128, dtype=np.float32)
    in_maps = []
    for c in range(NCORES):
        cols = slice(c * ML, (c + 1) * ML)
        in_maps.append({
            "Ash": np.ascontiguousarray(A[:, cols]),
            "Xf": X,
            "Xloc": np.ascontiguousarray(X[cols, :]),
            "WrT": WrT, "WqT": WqT, "WmT": WmT,
            "brel": brel, "bmlp": bmlp, "I128": I128_np,
        })
    return in_maps


def kernel(X, A, W_rel, b_rel, W_root, W_mlp, b_mlp):
    global last_exec_time_ns
    if "nc" not in _cache:
        _cache["nc"] = _build()
    nc = _cache["nc"]
    in_maps = _prep_inputs(X, A, W_rel, b_rel, W_root, W_mlp, b_mlp)
    trace = os.environ.get("BENCH_TRACE", "0") == "1"
    res = run_bass_kernel_spmd(nc, in_maps, list(range(NCORES)), trace=trace)
    last_exec_time_ns = res.exec_time_ns
    S = np.concatenate([res.results[c]["S_out"] for c in range(NCORES)], axis=0)
    losses = res.results[0]["losses"]
    return (S.astype(np.float32),
            np.float32(losses[0, 0]),
            np.float32(losses[0, 1]))
